# revision 3
# baseline (speedup 1.0000x reference)
"""Trainium2 Bass kernel for nn_DisRNNCellNet (time-decayed LSTM + noisy-OR).

Data-parallel over 8 NeuronCores: bsize 4096 -> 512/core = 4096 flat samples
per core (incl. 8 nodules). Per core a 32-step LSTM (hid=64) runs with
features on SBUF partitions and samples on the free dim.

Layout: samples split in halves A (0:2048) and B (2048:4096). Every
elementwise tile is [128, 2048] fp16 with rows 0:64 = half A, rows 64:128 =
half B, so all DVE ops run full-width with matching start partitions.

Engine balance (ACT is the bottleneck engine):
  - gate preacts per 1024-sample chunk, per gate X in {I,G,F,O}: one PSUM
    tile [128,1024] (2 banks; 4 gates = 8 banks, chunks reuse) filled by
    M=64 matmuls: rows 0:64 <- w_X.T @ xh_A, rows 64:128 <- w_X.T @ xh_B.
  - ACT: sig(I), tanh(G), sig(F), sig(O) from PSUM + part of tanh(c).
  - DVE: fd=sF*dc, c=ig+fd, h-muls, plus a deg-7 odd-polynomial tanh(c)
    chain for part of the batch (c stays in [-1.6,1.6]; poly fit on [-2,2]).
  - Pool (GpSimd): dc = c * dec (host-precomputed decay) and ig = sI*tG.

x is DMA'd one step ahead into ping-pong xh tiles ([x(64);h(64)] stacked
for K=128 fused matmuls). Final FC + noisy-OR pooling on-device.
"""

import math

import ml_dtypes
import numpy as np

import concourse.bass as bass
import concourse.mybir as mybir
import concourse.tile as tile
from concourse.bass_utils import run_bass_kernel_spmd

F16 = mybir.dt.float16
F32 = mybir.dt.float32
AF = mybir.ActivationFunctionType
ALU = mybir.AluOpType

STEP, BSIZE, NNOD, DIM, HID = 32, 4096, 8, 64, 64
NCORES = 8
BL = (BSIZE // NCORES) * NNOD  # 4096 flat samples per core
HALF = BL // 2  # 2048
NCH = 2  # chunks per step (psum working set = 8 banks per chunk)
CW = HALF // NCH  # 1024

# tanh(c) deg-7 odd polynomial on [-2,2]: t*(a1 + a3 t^2 + a5 t^4 + a7 t^6)
TANH_C = (0.9871337, -0.27664492, 0.058749425, -0.0053045610)
# widths (columns of each 1024-chunk) whose tanh(c) runs on the DVE chain;
# the rest goes through ACT. Tune against the engine-occupancy model.
TC_W = (1024, 0)
# ig = sig(I)*tanh(G) on Pool (True) or DVE (False), per chunk
IG_POOL = (True, True)

LAST_RESULT = None


def _split_multiwaits(nc, max_waits=1):
    """walrus in this env rejects >1 sem wait per instruction ("Too many
    sync wait commands"); split extras onto single-wait NoOps."""
    for bb in nc.main_func.blocks:
        out = []
        for ins in bb.instructions:
            si = ins.sync_info
            if si is not None and len(si.on_wait) > max_waits:
                waits = list(si.on_wait)
                for j, w in enumerate(waits[:-max_waits]):
                    out.append(
                        mybir.InstNoOp(
                            name=f"{ins.name}-wsplit{j}",
                            engine=ins.engine,
                            ins=[],
                            outs=[],
                            sync_info=mybir.SyncInfo(on_wait=[w], on_update=[]),
                        )
                    )
                ins.sync_info = mybir.SyncInfo(
                    on_wait=waits[-max_waits:], on_update=list(si.on_update)
                )
            out.append(ins)
        bb.instructions = out


def _build(fc2_b: float, k_base: float):
    nc = bass.Bass(target_bir_lowering=False)
    x_d = nc.declare_dram_parameter("x", [STEP, DIM, BL], F16, isOutput=False)
    dec_d = nc.declare_dram_parameter("dec", [STEP, 128, HALF], F16, isOutput=False)
    wi_d = nc.declare_dram_parameter("wi", [128, HID], F16, isOutput=False)
    wf_d = nc.declare_dram_parameter("wf", [128, HID], F16, isOutput=False)
    wg_d = nc.declare_dram_parameter("wg", [128, HID], F16, isOutput=False)
    wo_d = nc.declare_dram_parameter("wo", [128, HID], F16, isOutput=False)
    bi_d = nc.declare_dram_parameter("bi", [128, 1], F32, isOutput=False)
    bf_d = nc.declare_dram_parameter("bf", [128, 1], F32, isOutput=False)
    bg_d = nc.declare_dram_parameter("bg", [128, 1], F32, isOutput=False)
    bo_d = nc.declare_dram_parameter("bo", [128, 1], F32, isOutput=False)
    fc2_d = nc.declare_dram_parameter("fc2w", [HID, 1], F16, isOutput=False)
    out_d = nc.declare_dram_parameter("out", [1, BSIZE // NCORES], F32, isOutput=True)

    a1, a3, a5, a7 = TANH_C

    with tile.TileContext(nc) as tc:
        with (
            tc.tile_pool(name="const", bufs=1) as const,
            tc.tile_pool(name="decp", bufs=2) as decp,
            tc.tile_pool(name="work", bufs=2) as work,
            tc.tile_pool(name="psum", bufs=1, space="PSUM") as psum,
        ):
            wgt = {}
            for g, dr in [("i", wi_d), ("f", wf_d), ("g", wg_d), ("o", wo_d)]:
                wgt[g] = const.tile([128, HID], F16, tag=f"w{g}", name=f"w{g}")
                nc.sync.dma_start(out=wgt[g][:], in_=dr[:])
            bia = {}
            for g, dr in [("i", bi_d), ("f", bf_d), ("g", bg_d), ("o", bo_d)]:
                bia[g] = const.tile([128, 1], F32, tag=f"b{g}", name=f"b{g}")
                nc.sync.dma_start(out=bia[g][:], in_=dr[:])
            fc2 = const.tile([HID, 1], F16, tag="fc2", name="fc2")
            nc.sync.dma_start(out=fc2[:], in_=fc2_d[:])

            # ping-pong [x; h] tiles per half: rows 0:64 x_t, rows 64:128 h
            xh = [
                [
                    const.tile([128, HALF], F16, tag=f"xh{q}{p}", name=f"xh{q}{p}")
                    for p in range(2)
                ]
                for q in range(2)
            ]
            c2 = const.tile([128, HALF], F16, tag="c2", name="c2")
            nc.vector.memset(xh[0][0][HID:128, :], 0.0)
            nc.vector.memset(xh[1][0][HID:128, :], 0.0)
            nc.vector.memset(c2[:], 0.0)
            # x(0) into xh[*][0]
            nc.sync.dma_start(out=xh[0][0][0:DIM, :], in_=x_d[0, :, bass.ts(0, HALF)])
            nc.sync.dma_start(out=xh[1][0][0:DIM, :], in_=x_d[0, :, bass.ts(1, HALF)])
            dec0 = decp.tile([128, HALF], F16, tag="dec", name="dec0")
            nc.sync.dma_start(out=dec0[:], in_=dec_d[0])

            hfA = const.tile([HID, HALF], F16, tag="hfA", name="hfA")
            hfB = const.tile([HID, HALF], F16, tag="hfB", name="hfB")

            dect = dec0
            for t in range(STEP):
                par = t % 2
                xa, xb = xh[0][par], xh[1][par]
                na, nb = xh[0][1 - par], xh[1][1 - par]
                last = t == STEP - 1

                # prefetch x(t+1) and dec(t+1)
                if not last:
                    nc.sync.dma_start(
                        out=na[0:DIM, :], in_=x_d[t + 1, :, bass.ts(0, HALF)]
                    )
                    nc.sync.dma_start(
                        out=nb[0:DIM, :], in_=x_d[t + 1, :, bass.ts(1, HALF)]
                    )
                    decn = decp.tile([128, HALF], F16, tag="dec", name=f"dec{t + 1}")
                    nc.sync.dma_start(out=decn[:], in_=dec_d[t + 1])

                sI = work.tile([128, HALF], F16, tag="sI", name="sI")
                tG = work.tile([128, HALF], F16, tag="tG", name="tG")
                sF = work.tile([128, HALF], F16, tag="sF", name="sF")
                sO = work.tile([128, HALF], F16, tag="sO", name="sO")
                dc = work.tile([128, HALF], F16, tag="dc", name="dc")
                ig = work.tile([128, HALF], F16, tag="ig", name="ig")
                fd = work.tile([128, HALF], F16, tag="fd", name="fd")
                tch = work.tile([128, HALF], F16, tag="tch", name="tch")
                tp = work.tile([128, HALF], F16, tag="tp", name="tp")
                pp = work.tile([128, HALF], F16, tag="pp", name="pp")

                for ch in range(NCH):
                    cs = bass.ds(ch * CW, CW)
                    # decay * c on Pool (off the DVE critical path)
                    nc.gpsimd.tensor_mul(dc[:, cs], c2[:, cs], dect[:, cs])

                    # gate order matters: I,G first (feed ig), then F, O
                    for g, act, sbuf_out in (
                        ("i", AF.Sigmoid, sI),
                        ("g", AF.Tanh, tG),
                        ("f", AF.Sigmoid, sF),
                        ("o", AF.Sigmoid, sO),
                    ):
                        p = psum.tile([128, CW], F32, tag=f"p{g}", name=f"p{g}{ch}")
                        for j in range(CW // 512):
                            js = bass.ds(ch * CW + j * 512, 512)
                            ps = bass.ts(j, 512)
                            nc.tensor.matmul(
                                p[0:HID, ps], wgt[g][:], xa[:, js],
                                start=True, stop=True,
                            )
                            nc.tensor.matmul(
                                p[HID:128, ps], wgt[g][:], xb[:, js],
                                start=True, stop=True,
                            )
                        nc.scalar.activation(
                            sbuf_out[:, cs], p[:], act, bias=bia[g][:]
                        )

                    if IG_POOL[ch]:
                        nc.gpsimd.tensor_mul(ig[:, cs], sI[:, cs], tG[:, cs])
                    else:
                        nc.vector.tensor_mul(ig[:, cs], sI[:, cs], tG[:, cs])
                    nc.vector.tensor_mul(fd[:, cs], sF[:, cs], dc[:, cs])
                    nc.vector.tensor_add(c2[:, cs], ig[:, cs], fd[:, cs])

                    # tanh(c): DVE deg-7 chain on the first TC_W[ch] columns,
                    # ACT on the rest
                    wd = TC_W[ch]
                    if wd > 0:
                        cd = bass.ds(ch * CW, wd)
                        nc.vector.tensor_mul(tp[:, cd], c2[:, cd], c2[:, cd])
                        nc.vector.tensor_scalar(
                            out=pp[:, cd], in0=tp[:, cd],
                            scalar1=a7, scalar2=a5, op0=ALU.mult, op1=ALU.add,
                        )
                        nc.vector.tensor_mul(pp[:, cd], pp[:, cd], tp[:, cd])
                        nc.vector.tensor_scalar(
                            out=pp[:, cd], in0=pp[:, cd],
                            scalar1=a3, scalar2=None, op0=ALU.add,
                        )
                        nc.vector.tensor_mul(pp[:, cd], pp[:, cd], tp[:, cd])
                        nc.vector.tensor_scalar(
                            out=pp[:, cd], in0=pp[:, cd],
                            scalar1=a1, scalar2=None, op0=ALU.add,
                        )
                        nc.vector.tensor_mul(tch[:, cd], pp[:, cd], c2[:, cd])
                    if wd < CW:
                        ca = bass.ds(ch * CW + wd, CW - wd)
                        nc.scalar.activation(tch[:, ca], c2[:, ca], AF.Tanh)

                    # h = sig(o) * tanh(c); A-half rows shift 0:64 -> 64:128
                    ha = na[HID:128, cs] if not last else hfA[:, cs]
                    hb = nb[HID:128, cs] if not last else hfB[:, cs]
                    nc.vector.tensor_mul(ha, sO[0:HID, cs], tch[0:HID, cs])
                    nc.vector.tensor_mul(hb, sO[HID:128, cs], tch[HID:128, cs])
                dect = decn if not last else None

            # ---- final: q = 1 - sigmoid(h@w + b), noisy-OR over nodules ----
            nb2 = const.tile([1, 1], F32, tag="nb2", name="nb2")
            nc.vector.memset(nb2[:], -fc2_b)
            qall = const.tile([1, BL], F32, tag="qall", name="qall")
            for q, hf in ((0, hfA), (1, hfB)):
                for j in range(HALF // 512):
                    js = bass.ts(j, 512)
                    pz = psum.tile([1, 512], F32, tag="pi", name=f"pz{q}{j}")
                    nc.tensor.matmul(
                        pz[:], fc2[:], hf[:, js], start=True, stop=True
                    )
                    nc.scalar.activation(
                        qall[0:1, bass.ds(q * HALF + j * 512, 512)],
                        pz[:],
                        AF.Sigmoid,
                        scale=-1.0,
                        bias=nb2[:],
                    )
            # product over the 8 nodules (innermost in sample order)
            q3 = qall[0:1].rearrange("p (b n) -> p b n", n=NNOD)
            t1 = const.tile([1, BL // 2], F32, tag="t1", name="t1")
            t13 = t1[0:1].rearrange("p (b n) -> p b n", n=4)
            nc.vector.tensor_mul(t13[:, :, :], q3[:, :, 0:4], q3[:, :, 4:8])
            t2 = const.tile([1, BL // 4], F32, tag="t2", name="t2")
            t23 = t2[0:1].rearrange("p (b n) -> p b n", n=2)
            nc.vector.tensor_mul(t23[:, :, :], t13[:, :, 0:2], t13[:, :, 2:4])
            t3 = const.tile([1, BL // 8], F32, tag="t3", name="t3")
            t33 = t3[0:1].rearrange("p (b n) -> p b n", n=1)
            nc.vector.tensor_mul(t33[:, :, :], t23[:, :, 0:1], t23[:, :, 1:2])
            pred = const.tile([1, BSIZE // NCORES], F32, tag="pred", name="pred")
            nc.vector.tensor_scalar(
                out=pred[:],
                in0=t3[:],
                scalar1=-k_base,
                scalar2=1.0,
                op0=ALU.mult,
                op1=ALU.add,
            )
            nc.sync.dma_start(out=out_d[:], in_=pred[:])

    _split_multiwaits(nc)
    return nc


def kernel(input, time_dis, w_ih, w_hh, b_ih, b_hh, fc2_w, fc2_b, baseline):
    input = np.asarray(input, dtype=np.float32)
    time_dis = np.asarray(time_dis, dtype=np.float32)
    w_ih = np.asarray(w_ih, dtype=np.float32)
    w_hh = np.asarray(w_hh, dtype=np.float32)
    b_ih = np.asarray(b_ih, dtype=np.float32)
    b_hh = np.asarray(b_hh, dtype=np.float32)
    fc2_w = np.asarray(fc2_w, dtype=np.float32)
    fc2_b = np.asarray(fc2_b, dtype=np.float32)
    baseline = np.asarray(baseline, dtype=np.float32)

    f16 = np.float16
    bper = BSIZE // NCORES  # 512

    # gates^T = W^T.T @ [x;h], W = [w_ih | w_hh]  [256, 128]
    W = np.concatenate([w_ih, w_hh], axis=1)  # [256, 128]
    lhsT = np.ascontiguousarray(W.T)  # [128, 256] cols: i(0:64) f g o
    wi = np.ascontiguousarray(lhsT[:, 0:64]).astype(f16)
    wf = np.ascontiguousarray(lhsT[:, 64:128]).astype(f16)
    wg = np.ascontiguousarray(lhsT[:, 128:192]).astype(f16)
    wo = np.ascontiguousarray(lhsT[:, 192:256]).astype(f16)
    bias = (b_ih + b_hh).astype(np.float32)
    bi = np.ascontiguousarray(np.tile(bias[0:64], 2)[:, None])
    bfg = np.ascontiguousarray(np.tile(bias[64:128], 2)[:, None])
    bg = np.ascontiguousarray(np.tile(bias[128:192], 2)[:, None])
    bo = np.ascontiguousarray(np.tile(bias[192:256], 2)[:, None])
    fc2w = np.ascontiguousarray(fc2_w.reshape(1, HID).T).astype(f16)  # [64,1]
    k_base = float(1.0 - 1.0 / (1.0 + math.exp(-float(baseline[0]))))

    nc = _build(float(fc2_b[0]), k_base)

    in_maps = []
    for k in range(NCORES):
        bs = slice(k * bper, (k + 1) * bper)
        xs = input[:, bs].reshape(STEP, BL, DIM)
        xs = np.ascontiguousarray(xs.transpose(0, 2, 1)).astype(f16)  # [S,64,BL]
        td = time_dis[bs]  # [512, 32]
        td_bn = np.repeat(td.T, NNOD, axis=1)  # [32, 4096] sample-major
        td_used = np.concatenate([td_bn[:1], td_bn[:-1]], axis=0)
        dec = (1.0 / np.log(math.e + td_used)).astype(f16)  # [32, BL]
        # dec2[t, 0:64, j] = dec[t, j] (half A); [t, 64:128, j] = dec[t, HALF+j]
        dec2 = np.empty((STEP, 128, HALF), dtype=f16)
        dec2[:, 0:HID, :] = dec[:, None, 0:HALF]
        dec2[:, HID:128, :] = dec[:, None, HALF:BL]
        in_maps.append(
            {
                "x": xs,
                "dec": dec2,
                "wi": wi,
                "wf": wf,
                "wg": wg,
                "wo": wo,
                "bi": bi,
                "bf": bfg,
                "bg": bg,
                "bo": bo,
                "fc2w": fc2w,
            }
        )

    res = None
    last_err = None
    for _attempt in range(3):
        try:
            res = run_bass_kernel_spmd(nc, in_maps, list(range(NCORES)))
            break
        except Exception as e:  # transient NRT device errors recover on retry
            last_err = e
    if res is None:
        raise last_err
    global LAST_RESULT
    LAST_RESULT = res
    out = np.concatenate(
        [np.asarray(res.results[k]["out"]).reshape(bper) for k in range(NCORES)]
    )
    return out.astype(np.float32)


# revision 4
# speedup vs baseline: 1.2883x; 1.2883x over previous
"""Trainium2 Bass kernel for nn_DisRNNCellNet (time-decayed LSTM + noisy-OR).

Data-parallel over 8 NeuronCores: bsize 4096 -> 512/core = 4096 flat samples
per core (incl. 8 nodules). Per core a 32-step LSTM (hid=64) runs with
features on SBUF partitions and samples on the free dim.

Layout: samples split in halves A (0:2048) and B (2048:4096). Every
elementwise tile is [128, 2048] fp16 with rows 0:64 = half A, rows 64:128 =
half B, so all DVE ops run full-width with matching start partitions.

Engine balance (ACT is the bottleneck engine):
  - gate preacts per 1024-sample chunk, per gate X in {I,G,F,O}: one PSUM
    tile [128,1024] (2 banks; 4 gates = 8 banks, chunks reuse) filled by
    M=64 matmuls: rows 0:64 <- w_X.T @ xh_A, rows 64:128 <- w_X.T @ xh_B.
  - ACT: sig(I), tanh(G), sig(F), sig(O) from PSUM + part of tanh(c).
  - DVE: fd=sF*dc, c=ig+fd, h-muls, plus a deg-7 odd-polynomial tanh(c)
    chain for part of the batch (c stays in [-1.6,1.6]; poly fit on [-2,2]).
  - Pool (GpSimd): dc = c * dec (host-precomputed decay) and ig = sI*tG.

x is DMA'd one step ahead into ping-pong xh tiles ([x(64);h(64)] stacked
for K=128 fused matmuls). Final FC + noisy-OR pooling on-device.
"""

import math

import ml_dtypes
import numpy as np

import concourse.bass as bass
import concourse.mybir as mybir
import concourse.tile as tile
from concourse.bass_utils import run_bass_kernel_spmd

F16 = mybir.dt.float16
F32 = mybir.dt.float32
AF = mybir.ActivationFunctionType
ALU = mybir.AluOpType

STEP, BSIZE, NNOD, DIM, HID = 32, 4096, 8, 64, 64
NCORES = 8
BL = (BSIZE // NCORES) * NNOD  # 4096 flat samples per core
HALF = BL // 2  # 2048
NCH = 2  # chunks per step (psum working set = 8 banks per chunk)
CW = HALF // NCH  # 1024

# tanh(c) deg-7 odd polynomial on [-2,2]: t*(a1 + a3 t^2 + a5 t^4 + a7 t^6)
TANH_C = (0.9871337, -0.27664492, 0.058749425, -0.0053045610)
# widths (columns of each 1024-chunk) whose tanh(c) runs on the DVE chain;
# the rest goes through ACT. Tune against the engine-occupancy model.
TC_W = (512, 512)
# ig = sig(I)*tanh(G) on Pool (True) or DVE (False), per chunk
IG_POOL = (False, False)

LAST_RESULT = None


def _split_multiwaits(nc, max_waits=1):
    """walrus in this env rejects >1 sem wait per instruction ("Too many
    sync wait commands"); split extras onto single-wait NoOps."""
    for bb in nc.main_func.blocks:
        out = []
        for ins in bb.instructions:
            si = ins.sync_info
            if si is not None and len(si.on_wait) > max_waits:
                waits = list(si.on_wait)
                for j, w in enumerate(waits[:-max_waits]):
                    out.append(
                        mybir.InstNoOp(
                            name=f"{ins.name}-wsplit{j}",
                            engine=ins.engine,
                            ins=[],
                            outs=[],
                            sync_info=mybir.SyncInfo(on_wait=[w], on_update=[]),
                        )
                    )
                ins.sync_info = mybir.SyncInfo(
                    on_wait=waits[-max_waits:], on_update=list(si.on_update)
                )
            out.append(ins)
        bb.instructions = out


def _build(fc2_b: float, k_base: float):
    nc = bass.Bass(target_bir_lowering=False)
    x_d = nc.declare_dram_parameter("x", [STEP, DIM, BL], F16, isOutput=False)
    dec_d = nc.declare_dram_parameter("dec", [STEP, 128, HALF], F16, isOutput=False)
    wi_d = nc.declare_dram_parameter("wi", [128, HID], F16, isOutput=False)
    wf_d = nc.declare_dram_parameter("wf", [128, HID], F16, isOutput=False)
    wg_d = nc.declare_dram_parameter("wg", [128, HID], F16, isOutput=False)
    wo_d = nc.declare_dram_parameter("wo", [128, HID], F16, isOutput=False)
    bi_d = nc.declare_dram_parameter("bi", [128, 1], F32, isOutput=False)
    bf_d = nc.declare_dram_parameter("bf", [128, 1], F32, isOutput=False)
    bg_d = nc.declare_dram_parameter("bg", [128, 1], F32, isOutput=False)
    bo_d = nc.declare_dram_parameter("bo", [128, 1], F32, isOutput=False)
    fc2_d = nc.declare_dram_parameter("fc2w", [HID, 1], F16, isOutput=False)
    out_d = nc.declare_dram_parameter("out", [1, BSIZE // NCORES], F32, isOutput=True)

    a1, a3, a5, a7 = TANH_C

    with tile.TileContext(nc) as tc:
        with (
            tc.tile_pool(name="const", bufs=1) as const,
            tc.tile_pool(name="decp", bufs=2) as decp,
            tc.tile_pool(name="work", bufs=2) as work,
            tc.tile_pool(name="psum", bufs=1, space="PSUM") as psum,
        ):
            wgt = {}
            for g, dr in [("i", wi_d), ("f", wf_d), ("g", wg_d), ("o", wo_d)]:
                wgt[g] = const.tile([128, HID], F16, tag=f"w{g}", name=f"w{g}")
                nc.sync.dma_start(out=wgt[g][:], in_=dr[:])
            bia = {}
            for g, dr in [("i", bi_d), ("f", bf_d), ("g", bg_d), ("o", bo_d)]:
                bia[g] = const.tile([128, 1], F32, tag=f"b{g}", name=f"b{g}")
                nc.sync.dma_start(out=bia[g][:], in_=dr[:])
            fc2 = const.tile([HID, 1], F16, tag="fc2", name="fc2")
            nc.sync.dma_start(out=fc2[:], in_=fc2_d[:])

            # ping-pong [x; h] tiles per half: rows 0:64 x_t, rows 64:128 h
            xh = [
                [
                    const.tile([128, HALF], F16, tag=f"xh{q}{p}", name=f"xh{q}{p}")
                    for p in range(2)
                ]
                for q in range(2)
            ]
            c2 = const.tile([128, HALF], F16, tag="c2", name="c2")
            nc.vector.memset(xh[0][0][HID:128, :], 0.0)
            nc.vector.memset(xh[1][0][HID:128, :], 0.0)
            nc.vector.memset(c2[:], 0.0)
            # x(0) into xh[*][0]
            nc.sync.dma_start(out=xh[0][0][0:DIM, :], in_=x_d[0, :, bass.ts(0, HALF)])
            nc.sync.dma_start(out=xh[1][0][0:DIM, :], in_=x_d[0, :, bass.ts(1, HALF)])
            dec0 = decp.tile([128, HALF], F16, tag="dec", name="dec0")
            nc.sync.dma_start(out=dec0[:], in_=dec_d[0])

            hfA = const.tile([HID, HALF], F16, tag="hfA", name="hfA")
            hfB = const.tile([HID, HALF], F16, tag="hfB", name="hfB")

            dect = dec0
            for t in range(STEP):
                par = t % 2
                xa, xb = xh[0][par], xh[1][par]
                na, nb = xh[0][1 - par], xh[1][1 - par]
                last = t == STEP - 1

                # prefetch x(t+1) and dec(t+1)
                if not last:
                    nc.sync.dma_start(
                        out=na[0:DIM, :], in_=x_d[t + 1, :, bass.ts(0, HALF)]
                    )
                    nc.sync.dma_start(
                        out=nb[0:DIM, :], in_=x_d[t + 1, :, bass.ts(1, HALF)]
                    )
                    decn = decp.tile([128, HALF], F16, tag="dec", name=f"dec{t + 1}")
                    nc.sync.dma_start(out=decn[:], in_=dec_d[t + 1])

                sI = work.tile([128, HALF], F16, tag="sI", name="sI")
                tG = work.tile([128, HALF], F16, tag="tG", name="tG")
                sF = work.tile([128, HALF], F16, tag="sF", name="sF")
                sO = work.tile([128, HALF], F16, tag="sO", name="sO")
                dc = work.tile([128, HALF], F16, tag="dc", name="dc")
                ig = work.tile([128, HALF], F16, tag="ig", name="ig")
                fd = work.tile([128, HALF], F16, tag="fd", name="fd")
                tch = work.tile([128, HALF], F16, tag="tch", name="tch")
                tp = work.tile([128, HALF], F16, tag="tp", name="tp")
                pp = work.tile([128, HALF], F16, tag="pp", name="pp")

                for ch in range(NCH):
                    cs = bass.ds(ch * CW, CW)
                    # decay * c on Pool (off the DVE critical path)
                    nc.gpsimd.tensor_mul(dc[:, cs], c2[:, cs], dect[:, cs])

                    # gate order matters: I,G first (feed ig), then F, O
                    for g, act, sbuf_out in (
                        ("i", AF.Sigmoid, sI),
                        ("g", AF.Tanh, tG),
                        ("f", AF.Sigmoid, sF),
                        ("o", AF.Sigmoid, sO),
                    ):
                        p = psum.tile([128, CW], F32, tag=f"p{g}", name=f"p{g}{ch}")
                        for j in range(CW // 512):
                            js = bass.ds(ch * CW + j * 512, 512)
                            ps = bass.ts(j, 512)
                            nc.tensor.matmul(
                                p[0:HID, ps], wgt[g][:], xa[:, js],
                                start=True, stop=True,
                            )
                            nc.tensor.matmul(
                                p[HID:128, ps], wgt[g][:], xb[:, js],
                                start=True, stop=True,
                            )
                        nc.scalar.activation(
                            sbuf_out[:, cs], p[:], act, bias=bia[g][:]
                        )

                    if IG_POOL[ch]:
                        nc.gpsimd.tensor_mul(ig[:, cs], sI[:, cs], tG[:, cs])
                    else:
                        nc.vector.tensor_mul(ig[:, cs], sI[:, cs], tG[:, cs])
                    nc.vector.tensor_mul(fd[:, cs], sF[:, cs], dc[:, cs])
                    nc.vector.tensor_add(c2[:, cs], ig[:, cs], fd[:, cs])

                    # tanh(c): DVE deg-7 chain on the first TC_W[ch] columns,
                    # ACT on the rest
                    wd = TC_W[ch]
                    if wd > 0:
                        cd = bass.ds(ch * CW, wd)
                        nc.vector.tensor_mul(tp[:, cd], c2[:, cd], c2[:, cd])
                        nc.vector.tensor_scalar(
                            out=pp[:, cd], in0=tp[:, cd],
                            scalar1=a7, scalar2=a5, op0=ALU.mult, op1=ALU.add,
                        )
                        nc.vector.tensor_mul(pp[:, cd], pp[:, cd], tp[:, cd])
                        nc.vector.tensor_scalar(
                            out=pp[:, cd], in0=pp[:, cd],
                            scalar1=a3, scalar2=None, op0=ALU.add,
                        )
                        nc.vector.tensor_mul(pp[:, cd], pp[:, cd], tp[:, cd])
                        nc.vector.tensor_scalar(
                            out=pp[:, cd], in0=pp[:, cd],
                            scalar1=a1, scalar2=None, op0=ALU.add,
                        )
                        nc.vector.tensor_mul(tch[:, cd], pp[:, cd], c2[:, cd])
                    if wd < CW:
                        ca = bass.ds(ch * CW + wd, CW - wd)
                        nc.scalar.activation(tch[:, ca], c2[:, ca], AF.Tanh)

                    # h = sig(o) * tanh(c); A-half rows shift 0:64 -> 64:128
                    ha = na[HID:128, cs] if not last else hfA[:, cs]
                    hb = nb[HID:128, cs] if not last else hfB[:, cs]
                    nc.vector.tensor_mul(ha, sO[0:HID, cs], tch[0:HID, cs])
                    nc.vector.tensor_mul(hb, sO[HID:128, cs], tch[HID:128, cs])
                dect = decn if not last else None

            # ---- final: q = 1 - sigmoid(h@w + b), noisy-OR over nodules ----
            nb2 = const.tile([1, 1], F32, tag="nb2", name="nb2")
            nc.vector.memset(nb2[:], -fc2_b)
            qall = const.tile([1, BL], F32, tag="qall", name="qall")
            for q, hf in ((0, hfA), (1, hfB)):
                for j in range(HALF // 512):
                    js = bass.ts(j, 512)
                    pz = psum.tile([1, 512], F32, tag="pi", name=f"pz{q}{j}")
                    nc.tensor.matmul(
                        pz[:], fc2[:], hf[:, js], start=True, stop=True
                    )
                    nc.scalar.activation(
                        qall[0:1, bass.ds(q * HALF + j * 512, 512)],
                        pz[:],
                        AF.Sigmoid,
                        scale=-1.0,
                        bias=nb2[:],
                    )
            # product over the 8 nodules (innermost in sample order)
            q3 = qall[0:1].rearrange("p (b n) -> p b n", n=NNOD)
            t1 = const.tile([1, BL // 2], F32, tag="t1", name="t1")
            t13 = t1[0:1].rearrange("p (b n) -> p b n", n=4)
            nc.vector.tensor_mul(t13[:, :, :], q3[:, :, 0:4], q3[:, :, 4:8])
            t2 = const.tile([1, BL // 4], F32, tag="t2", name="t2")
            t23 = t2[0:1].rearrange("p (b n) -> p b n", n=2)
            nc.vector.tensor_mul(t23[:, :, :], t13[:, :, 0:2], t13[:, :, 2:4])
            t3 = const.tile([1, BL // 8], F32, tag="t3", name="t3")
            t33 = t3[0:1].rearrange("p (b n) -> p b n", n=1)
            nc.vector.tensor_mul(t33[:, :, :], t23[:, :, 0:1], t23[:, :, 1:2])
            pred = const.tile([1, BSIZE // NCORES], F32, tag="pred", name="pred")
            nc.vector.tensor_scalar(
                out=pred[:],
                in0=t3[:],
                scalar1=-k_base,
                scalar2=1.0,
                op0=ALU.mult,
                op1=ALU.add,
            )
            nc.sync.dma_start(out=out_d[:], in_=pred[:])

    _split_multiwaits(nc)
    return nc


def kernel(input, time_dis, w_ih, w_hh, b_ih, b_hh, fc2_w, fc2_b, baseline):
    input = np.asarray(input, dtype=np.float32)
    time_dis = np.asarray(time_dis, dtype=np.float32)
    w_ih = np.asarray(w_ih, dtype=np.float32)
    w_hh = np.asarray(w_hh, dtype=np.float32)
    b_ih = np.asarray(b_ih, dtype=np.float32)
    b_hh = np.asarray(b_hh, dtype=np.float32)
    fc2_w = np.asarray(fc2_w, dtype=np.float32)
    fc2_b = np.asarray(fc2_b, dtype=np.float32)
    baseline = np.asarray(baseline, dtype=np.float32)

    f16 = np.float16
    bper = BSIZE // NCORES  # 512

    # gates^T = W^T.T @ [x;h], W = [w_ih | w_hh]  [256, 128]
    W = np.concatenate([w_ih, w_hh], axis=1)  # [256, 128]
    lhsT = np.ascontiguousarray(W.T)  # [128, 256] cols: i(0:64) f g o
    wi = np.ascontiguousarray(lhsT[:, 0:64]).astype(f16)
    wf = np.ascontiguousarray(lhsT[:, 64:128]).astype(f16)
    wg = np.ascontiguousarray(lhsT[:, 128:192]).astype(f16)
    wo = np.ascontiguousarray(lhsT[:, 192:256]).astype(f16)
    bias = (b_ih + b_hh).astype(np.float32)
    bi = np.ascontiguousarray(np.tile(bias[0:64], 2)[:, None])
    bfg = np.ascontiguousarray(np.tile(bias[64:128], 2)[:, None])
    bg = np.ascontiguousarray(np.tile(bias[128:192], 2)[:, None])
    bo = np.ascontiguousarray(np.tile(bias[192:256], 2)[:, None])
    fc2w = np.ascontiguousarray(fc2_w.reshape(1, HID).T).astype(f16)  # [64,1]
    k_base = float(1.0 - 1.0 / (1.0 + math.exp(-float(baseline[0]))))

    nc = _build(float(fc2_b[0]), k_base)

    in_maps = []
    for k in range(NCORES):
        bs = slice(k * bper, (k + 1) * bper)
        xs = input[:, bs].reshape(STEP, BL, DIM)
        xs = np.ascontiguousarray(xs.transpose(0, 2, 1)).astype(f16)  # [S,64,BL]
        td = time_dis[bs]  # [512, 32]
        td_bn = np.repeat(td.T, NNOD, axis=1)  # [32, 4096] sample-major
        td_used = np.concatenate([td_bn[:1], td_bn[:-1]], axis=0)
        dec = (1.0 / np.log(math.e + td_used)).astype(f16)  # [32, BL]
        # dec2[t, 0:64, j] = dec[t, j] (half A); [t, 64:128, j] = dec[t, HALF+j]
        dec2 = np.empty((STEP, 128, HALF), dtype=f16)
        dec2[:, 0:HID, :] = dec[:, None, 0:HALF]
        dec2[:, HID:128, :] = dec[:, None, HALF:BL]
        in_maps.append(
            {
                "x": xs,
                "dec": dec2,
                "wi": wi,
                "wf": wf,
                "wg": wg,
                "wo": wo,
                "bi": bi,
                "bf": bfg,
                "bg": bg,
                "bo": bo,
                "fc2w": fc2w,
            }
        )

    res = None
    last_err = None
    for _attempt in range(3):
        try:
            res = run_bass_kernel_spmd(nc, in_maps, list(range(NCORES)))
            break
        except Exception as e:  # transient NRT device errors recover on retry
            last_err = e
    if res is None:
        raise last_err
    global LAST_RESULT
    LAST_RESULT = res
    out = np.concatenate(
        [np.asarray(res.results[k]["out"]).reshape(bper) for k in range(NCORES)]
    )
    return out.astype(np.float32)


# revision 7
# speedup vs baseline: 1.2968x; 1.0067x over previous
"""Trainium2 Bass kernel for nn_DisRNNCellNet (time-decayed LSTM + noisy-OR).

Data-parallel over 8 NeuronCores: bsize 4096 -> 512/core = 4096 flat samples
per core (incl. 8 nodules). Per core a 32-step LSTM (hid=64) runs with
features on SBUF partitions and samples on the free dim.

Layout: samples split in halves A (0:2048) and B (2048:4096). Every
elementwise tile is [128, 2048] fp16 with rows 0:64 = half A, rows 64:128 =
half B, so all DVE ops run full-width with matching start partitions.

Engine balance (ACT is the bottleneck engine):
  - gate preacts per 1024-sample chunk, per gate X in {I,G,F,O}: one PSUM
    tile [128,1024] (2 banks; 4 gates = 8 banks, chunks reuse) filled by
    M=64 matmuls: rows 0:64 <- w_X.T @ xh_A, rows 64:128 <- w_X.T @ xh_B.
  - ACT: sig(I), tanh(G), sig(F), sig(O) from PSUM + part of tanh(c).
  - DVE: fd=sF*dc, c=ig+fd, h-muls, plus a deg-7 odd-polynomial tanh(c)
    chain for part of the batch (c stays in [-1.6,1.6]; poly fit on [-2,2]).
  - Pool (GpSimd): dc = c * dec (host-precomputed decay) and ig = sI*tG.

x is DMA'd one step ahead into ping-pong xh tiles ([x(64);h(64)] stacked
for K=128 fused matmuls). Final FC + noisy-OR pooling on-device.
"""

import math

import ml_dtypes
import numpy as np

import concourse.bass as bass
import concourse.mybir as mybir
import concourse.tile as tile
from concourse.bass_utils import run_bass_kernel_spmd

F16 = mybir.dt.float16
F32 = mybir.dt.float32
AF = mybir.ActivationFunctionType
ALU = mybir.AluOpType

STEP, BSIZE, NNOD, DIM, HID = 32, 4096, 8, 64, 64
NCORES = 8
BL = (BSIZE // NCORES) * NNOD  # 4096 flat samples per core
HALF = BL // 2  # 2048
NCH = 2  # chunks per step (psum working set = 8 banks per chunk)
CW = HALF // NCH  # 1024

# tanh(c) deg-5 odd polynomial on [-1.8,1.8]: t*(a1 + a3 t^2 + a5 t^4)
TANH_C5 = (0.96838165, -0.22814101, 0.028562382)
# per chunk-lane: tanh(c) on the DVE polynomial chain (True) or ACT (False).
# Lane 0 paces the whole pipeline (engines execute in program order), so it
# stays on ACT with minimum latency; lane 1 rides in lane 0's shadow and
# absorbs the DVE chain + Pool ig.
TC_DVE = (False, True)
# ig = sig(I)*tanh(G) on Pool (True) or DVE (False), per chunk
IG_POOL = (False, True)

LAST_RESULT = None


def _split_multiwaits(nc, max_waits=1):
    """walrus in this env rejects >1 sem wait per instruction ("Too many
    sync wait commands"); split extras onto single-wait NoOps."""
    for bb in nc.main_func.blocks:
        out = []
        for ins in bb.instructions:
            si = ins.sync_info
            if si is not None and len(si.on_wait) > max_waits:
                waits = list(si.on_wait)
                for j, w in enumerate(waits[:-max_waits]):
                    out.append(
                        mybir.InstNoOp(
                            name=f"{ins.name}-wsplit{j}",
                            engine=ins.engine,
                            ins=[],
                            outs=[],
                            sync_info=mybir.SyncInfo(on_wait=[w], on_update=[]),
                        )
                    )
                ins.sync_info = mybir.SyncInfo(
                    on_wait=waits[-max_waits:], on_update=list(si.on_update)
                )
            out.append(ins)
        bb.instructions = out


def _build(fc2_b: float, k_base: float):
    nc = bass.Bass(target_bir_lowering=False)
    x_d = nc.declare_dram_parameter("x", [STEP, DIM, BL], F16, isOutput=False)
    dec_d = nc.declare_dram_parameter("dec", [STEP, 128, HALF], F16, isOutput=False)
    wi_d = nc.declare_dram_parameter("wi", [128, HID], F16, isOutput=False)
    wf_d = nc.declare_dram_parameter("wf", [128, HID], F16, isOutput=False)
    wg_d = nc.declare_dram_parameter("wg", [128, HID], F16, isOutput=False)
    wo_d = nc.declare_dram_parameter("wo", [128, HID], F16, isOutput=False)
    bi_d = nc.declare_dram_parameter("bi", [128, 1], F32, isOutput=False)
    bf_d = nc.declare_dram_parameter("bf", [128, 1], F32, isOutput=False)
    bg_d = nc.declare_dram_parameter("bg", [128, 1], F32, isOutput=False)
    bo_d = nc.declare_dram_parameter("bo", [128, 1], F32, isOutput=False)
    fc2_d = nc.declare_dram_parameter("fc2w", [HID, 1], F16, isOutput=False)
    out_d = nc.declare_dram_parameter("out", [1, BSIZE // NCORES], F32, isOutput=True)

    a1, a3, a5 = TANH_C5

    with tile.TileContext(nc) as tc:
        with (
            tc.tile_pool(name="const", bufs=1) as const,
            tc.tile_pool(name="decp", bufs=2) as decp,
            tc.tile_pool(name="work", bufs=2) as work,
            tc.tile_pool(name="psum", bufs=1, space="PSUM") as psum,
        ):
            wgt = {}
            for g, dr in [("i", wi_d), ("f", wf_d), ("g", wg_d), ("o", wo_d)]:
                wgt[g] = const.tile([128, HID], F16, tag=f"w{g}", name=f"w{g}")
                nc.sync.dma_start(out=wgt[g][:], in_=dr[:])
            bia = {}
            for g, dr in [("i", bi_d), ("f", bf_d), ("g", bg_d), ("o", bo_d)]:
                bia[g] = const.tile([128, 1], F32, tag=f"b{g}", name=f"b{g}")
                nc.sync.dma_start(out=bia[g][:], in_=dr[:])
            fc2 = const.tile([HID, 1], F16, tag="fc2", name="fc2")
            nc.sync.dma_start(out=fc2[:], in_=fc2_d[:])

            # ping-pong [x; h] tiles per half: rows 0:64 x_t, rows 64:128 h
            xh = [
                [
                    const.tile([128, HALF], F16, tag=f"xh{q}{p}", name=f"xh{q}{p}")
                    for p in range(2)
                ]
                for q in range(2)
            ]
            c2 = const.tile([128, HALF], F16, tag="c2", name="c2")
            nc.vector.memset(xh[0][0][HID:128, :], 0.0)
            nc.vector.memset(xh[1][0][HID:128, :], 0.0)
            nc.vector.memset(c2[:], 0.0)
            # x(0) into xh[*][0]
            nc.sync.dma_start(out=xh[0][0][0:DIM, :], in_=x_d[0, :, bass.ts(0, HALF)])
            nc.sync.dma_start(out=xh[1][0][0:DIM, :], in_=x_d[0, :, bass.ts(1, HALF)])
            dec0 = decp.tile([128, HALF], F16, tag="dec", name="dec0")
            nc.sync.dma_start(out=dec0[:], in_=dec_d[0])

            hfA = const.tile([HID, HALF], F16, tag="hfA", name="hfA")
            hfB = const.tile([HID, HALF], F16, tag="hfB", name="hfB")

            dect = dec0
            for t in range(STEP):
                par = t % 2
                xa, xb = xh[0][par], xh[1][par]
                na, nb = xh[0][1 - par], xh[1][1 - par]
                last = t == STEP - 1

                # prefetch x(t+1) and dec(t+1)
                if not last:
                    nc.sync.dma_start(
                        out=na[0:DIM, :], in_=x_d[t + 1, :, bass.ts(0, HALF)]
                    )
                    nc.sync.dma_start(
                        out=nb[0:DIM, :], in_=x_d[t + 1, :, bass.ts(1, HALF)]
                    )
                    decn = decp.tile([128, HALF], F16, tag="dec", name=f"dec{t + 1}")
                    nc.sync.dma_start(out=decn[:], in_=dec_d[t + 1])

                sI = work.tile([128, HALF], F16, tag="sI", name="sI")
                tG = work.tile([128, HALF], F16, tag="tG", name="tG")
                sF = work.tile([128, HALF], F16, tag="sF", name="sF")
                sO = work.tile([128, HALF], F16, tag="sO", name="sO")
                dc = work.tile([128, HALF], F16, tag="dc", name="dc")
                ig = work.tile([128, HALF], F16, tag="ig", name="ig")
                fd = work.tile([128, HALF], F16, tag="fd", name="fd")
                tch = work.tile([128, HALF], F16, tag="tch", name="tch")
                tp = work.tile([128, HALF], F16, tag="tp", name="tp")
                pp = work.tile([128, HALF], F16, tag="pp", name="pp")

                for ch in range(NCH):
                    cs = bass.ds(ch * CW, CW)
                    # decay * c on Pool (off the DVE critical path)
                    nc.gpsimd.tensor_mul(dc[:, cs], c2[:, cs], dect[:, cs])

                    # gate order matters: I,G first (feed ig), then F, O
                    for g, act, sbuf_out in (
                        ("i", AF.Sigmoid, sI),
                        ("g", AF.Tanh, tG),
                        ("f", AF.Sigmoid, sF),
                        ("o", AF.Sigmoid, sO),
                    ):
                        p = psum.tile([128, CW], F32, tag=f"p{g}", name=f"p{g}{ch}")
                        for j in range(CW // 512):
                            js = bass.ds(ch * CW + j * 512, 512)
                            ps = bass.ts(j, 512)
                            nc.tensor.matmul(
                                p[0:HID, ps], wgt[g][:], xa[:, js],
                                start=True, stop=True,
                            )
                        for j in range(CW // 512):
                            js = bass.ds(ch * CW + j * 512, 512)
                            ps = bass.ts(j, 512)
                            nc.tensor.matmul(
                                p[HID:128, ps], wgt[g][:], xb[:, js],
                                start=True, stop=True,
                            )
                        nc.scalar.activation(
                            sbuf_out[:, cs], p[:], act, bias=bia[g][:]
                        )

                    if IG_POOL[ch]:
                        nc.gpsimd.tensor_mul(ig[:, cs], sI[:, cs], tG[:, cs])
                    else:
                        nc.vector.tensor_mul(ig[:, cs], sI[:, cs], tG[:, cs])
                    nc.vector.tensor_mul(fd[:, cs], sF[:, cs], dc[:, cs])
                    nc.vector.tensor_add(c2[:, cs], ig[:, cs], fd[:, cs])

                    # tanh(c) + h-muls at 512 granularity so the next step's
                    # PE burst (which consumes h slice-by-slice) starts early
                    for j in range(CW // 512):
                        cd = bass.ds(ch * CW + j * 512, 512)
                        if TC_DVE[ch]:
                            nc.vector.tensor_mul(tp[:, cd], c2[:, cd], c2[:, cd])
                            nc.vector.tensor_scalar(
                                out=pp[:, cd], in0=tp[:, cd],
                                scalar1=a5, scalar2=a3, op0=ALU.mult, op1=ALU.add,
                            )
                            nc.vector.tensor_mul(pp[:, cd], pp[:, cd], tp[:, cd])
                            nc.vector.tensor_scalar(
                                out=pp[:, cd], in0=pp[:, cd],
                                scalar1=a1, scalar2=None, op0=ALU.add,
                            )
                            nc.vector.tensor_mul(tch[:, cd], pp[:, cd], c2[:, cd])
                        else:
                            nc.scalar.activation(tch[:, cd], c2[:, cd], AF.Tanh)
                        # h = sig(o)*tanh(c); A-half rows shift 0:64 -> 64:128
                        ha = na[HID:128, cd] if not last else hfA[:, cd]
                        hb = nb[HID:128, cd] if not last else hfB[:, cd]
                        nc.vector.tensor_mul(ha, sO[0:HID, cd], tch[0:HID, cd])
                        nc.vector.tensor_mul(hb, sO[HID:128, cd], tch[HID:128, cd])
                dect = decn if not last else None

            # ---- final: q = 1 - sigmoid(h@w + b), noisy-OR over nodules ----
            nb2 = const.tile([1, 1], F32, tag="nb2", name="nb2")
            nc.vector.memset(nb2[:], -fc2_b)
            qall = const.tile([1, BL], F32, tag="qall", name="qall")
            for q, hf in ((0, hfA), (1, hfB)):
                for j in range(HALF // 512):
                    js = bass.ts(j, 512)
                    pz = psum.tile([1, 512], F32, tag="pi", name=f"pz{q}{j}")
                    nc.tensor.matmul(
                        pz[:], fc2[:], hf[:, js], start=True, stop=True
                    )
                    nc.scalar.activation(
                        qall[0:1, bass.ds(q * HALF + j * 512, 512)],
                        pz[:],
                        AF.Sigmoid,
                        scale=-1.0,
                        bias=nb2[:],
                    )
            # product over the 8 nodules (innermost in sample order)
            q3 = qall[0:1].rearrange("p (b n) -> p b n", n=NNOD)
            t1 = const.tile([1, BL // 2], F32, tag="t1", name="t1")
            t13 = t1[0:1].rearrange("p (b n) -> p b n", n=4)
            nc.vector.tensor_mul(t13[:, :, :], q3[:, :, 0:4], q3[:, :, 4:8])
            t2 = const.tile([1, BL // 4], F32, tag="t2", name="t2")
            t23 = t2[0:1].rearrange("p (b n) -> p b n", n=2)
            nc.vector.tensor_mul(t23[:, :, :], t13[:, :, 0:2], t13[:, :, 2:4])
            t3 = const.tile([1, BL // 8], F32, tag="t3", name="t3")
            t33 = t3[0:1].rearrange("p (b n) -> p b n", n=1)
            nc.vector.tensor_mul(t33[:, :, :], t23[:, :, 0:1], t23[:, :, 1:2])
            pred = const.tile([1, BSIZE // NCORES], F32, tag="pred", name="pred")
            nc.vector.tensor_scalar(
                out=pred[:],
                in0=t3[:],
                scalar1=-k_base,
                scalar2=1.0,
                op0=ALU.mult,
                op1=ALU.add,
            )
            nc.sync.dma_start(out=out_d[:], in_=pred[:])

    _split_multiwaits(nc)
    return nc


def kernel(input, time_dis, w_ih, w_hh, b_ih, b_hh, fc2_w, fc2_b, baseline):
    input = np.asarray(input, dtype=np.float32)
    time_dis = np.asarray(time_dis, dtype=np.float32)
    w_ih = np.asarray(w_ih, dtype=np.float32)
    w_hh = np.asarray(w_hh, dtype=np.float32)
    b_ih = np.asarray(b_ih, dtype=np.float32)
    b_hh = np.asarray(b_hh, dtype=np.float32)
    fc2_w = np.asarray(fc2_w, dtype=np.float32)
    fc2_b = np.asarray(fc2_b, dtype=np.float32)
    baseline = np.asarray(baseline, dtype=np.float32)

    f16 = np.float16
    bper = BSIZE // NCORES  # 512

    # gates^T = W^T.T @ [x;h], W = [w_ih | w_hh]  [256, 128]
    W = np.concatenate([w_ih, w_hh], axis=1)  # [256, 128]
    lhsT = np.ascontiguousarray(W.T)  # [128, 256] cols: i(0:64) f g o
    wi = np.ascontiguousarray(lhsT[:, 0:64]).astype(f16)
    wf = np.ascontiguousarray(lhsT[:, 64:128]).astype(f16)
    wg = np.ascontiguousarray(lhsT[:, 128:192]).astype(f16)
    wo = np.ascontiguousarray(lhsT[:, 192:256]).astype(f16)
    bias = (b_ih + b_hh).astype(np.float32)
    bi = np.ascontiguousarray(np.tile(bias[0:64], 2)[:, None])
    bfg = np.ascontiguousarray(np.tile(bias[64:128], 2)[:, None])
    bg = np.ascontiguousarray(np.tile(bias[128:192], 2)[:, None])
    bo = np.ascontiguousarray(np.tile(bias[192:256], 2)[:, None])
    fc2w = np.ascontiguousarray(fc2_w.reshape(1, HID).T).astype(f16)  # [64,1]
    k_base = float(1.0 - 1.0 / (1.0 + math.exp(-float(baseline[0]))))

    nc = _build(float(fc2_b[0]), k_base)

    in_maps = []
    for k in range(NCORES):
        bs = slice(k * bper, (k + 1) * bper)
        xs = input[:, bs].reshape(STEP, BL, DIM)
        xs = np.ascontiguousarray(xs.transpose(0, 2, 1)).astype(f16)  # [S,64,BL]
        td = time_dis[bs]  # [512, 32]
        td_bn = np.repeat(td.T, NNOD, axis=1)  # [32, 4096] sample-major
        td_used = np.concatenate([td_bn[:1], td_bn[:-1]], axis=0)
        dec = (1.0 / np.log(math.e + td_used)).astype(f16)  # [32, BL]
        # dec2[t, 0:64, j] = dec[t, j] (half A); [t, 64:128, j] = dec[t, HALF+j]
        dec2 = np.empty((STEP, 128, HALF), dtype=f16)
        dec2[:, 0:HID, :] = dec[:, None, 0:HALF]
        dec2[:, HID:128, :] = dec[:, None, HALF:BL]
        in_maps.append(
            {
                "x": xs,
                "dec": dec2,
                "wi": wi,
                "wf": wf,
                "wg": wg,
                "wo": wo,
                "bi": bi,
                "bf": bfg,
                "bg": bg,
                "bo": bo,
                "fc2w": fc2w,
            }
        )

    res = None
    last_err = None
    for _attempt in range(3):
        try:
            res = run_bass_kernel_spmd(nc, in_maps, list(range(NCORES)))
            break
        except Exception as e:  # transient NRT device errors recover on retry
            last_err = e
    if res is None:
        raise last_err
    global LAST_RESULT
    LAST_RESULT = res
    out = np.concatenate(
        [np.asarray(res.results[k]["out"]).reshape(bper) for k in range(NCORES)]
    )
    return out.astype(np.float32)


# revision 9
# speedup vs baseline: 1.4116x; 1.0885x over previous
"""Trainium2 Bass kernel for nn_DisRNNCellNet (time-decayed LSTM + noisy-OR).

Data-parallel over 8 NeuronCores: bsize 4096 -> 512/core = 4096 flat samples
per core (incl. 8 nodules). Per core a 32-step LSTM (hid=64) runs with
features on SBUF partitions and samples on the free dim.

Layout: samples split in halves A (0:2048) and B (2048:4096). Every
elementwise tile is [128, 2048] fp16 with rows 0:64 = half A, rows 64:128 =
half B, so all DVE ops run full-width with matching start partitions.

Engine balance (ACT is the bottleneck engine):
  - gate preacts per 1024-sample chunk, per gate X in {I,G,F,O}: one PSUM
    tile [128,1024] (2 banks; 4 gates = 8 banks, chunks reuse) filled by
    M=64 matmuls: rows 0:64 <- w_X.T @ xh_A, rows 64:128 <- w_X.T @ xh_B.
  - ACT: sig(I), tanh(G), sig(F), sig(O) from PSUM + part of tanh(c).
  - DVE: fd=sF*dc, c=ig+fd, h-muls, plus a deg-7 odd-polynomial tanh(c)
    chain for part of the batch (c stays in [-1.6,1.6]; poly fit on [-2,2]).
  - Pool (GpSimd): dc = c * dec (host-precomputed decay) and ig = sI*tG.

x is DMA'd one step ahead into ping-pong xh tiles ([x(64);h(64)] stacked
for K=128 fused matmuls). Final FC + noisy-OR pooling on-device.
"""

import math

import ml_dtypes
import numpy as np

import concourse.bass as bass
import concourse.mybir as mybir
import concourse.tile as tile
from concourse.bass_utils import run_bass_kernel_spmd

F16 = mybir.dt.float16
F32 = mybir.dt.float32
AF = mybir.ActivationFunctionType
ALU = mybir.AluOpType

STEP, BSIZE, NNOD, DIM, HID = 32, 4096, 8, 64, 64
NCORES = 8
BL = (BSIZE // NCORES) * NNOD  # 4096 flat samples per core
HALF = BL // 2  # 2048
NCH = 2  # chunks per step (psum working set = 8 banks per chunk)
CW = HALF // NCH  # 1024

# tanh(c) deg-5 odd polynomial on [-1.8,1.8]: t*(a1 + a3 t^2 + a5 t^4)
TANH_C5 = (0.96838165, -0.22814101, 0.028562382)
# columns (of each 1024-wide lane) whose tanh(c) runs as a DVE polynomial
# chain; the first CW-TCW columns go through ACT. Balances ACT vs DVE.
TCW = 512
# ig = sig(I)*tanh(G) on Pool (True) or DVE (False)
IG_POOL = False

LAST_RESULT = None


def _split_multiwaits(nc, max_waits=1):
    """walrus in this env rejects >1 sem wait per instruction ("Too many
    sync wait commands"); split extras onto single-wait NoOps."""
    for bb in nc.main_func.blocks:
        out = []
        for ins in bb.instructions:
            si = ins.sync_info
            if si is not None and len(si.on_wait) > max_waits:
                waits = list(si.on_wait)
                for j, w in enumerate(waits[:-max_waits]):
                    out.append(
                        mybir.InstNoOp(
                            name=f"{ins.name}-wsplit{j}",
                            engine=ins.engine,
                            ins=[],
                            outs=[],
                            sync_info=mybir.SyncInfo(on_wait=[w], on_update=[]),
                        )
                    )
                ins.sync_info = mybir.SyncInfo(
                    on_wait=waits[-max_waits:], on_update=list(si.on_update)
                )
            out.append(ins)
        bb.instructions = out


def _build(fc2_b: float, k_base: float):
    nc = bass.Bass(target_bir_lowering=False)
    x_d = nc.declare_dram_parameter("x", [STEP, DIM, BL], F16, isOutput=False)
    dec_d = nc.declare_dram_parameter("dec", [STEP, 128, HALF], F16, isOutput=False)
    wi_d = nc.declare_dram_parameter("wi", [128, HID], F16, isOutput=False)
    wf_d = nc.declare_dram_parameter("wf", [128, HID], F16, isOutput=False)
    wg_d = nc.declare_dram_parameter("wg", [128, HID], F16, isOutput=False)
    wo_d = nc.declare_dram_parameter("wo", [128, HID], F16, isOutput=False)
    bi_d = nc.declare_dram_parameter("bi", [128, 1], F32, isOutput=False)
    bf_d = nc.declare_dram_parameter("bf", [128, 1], F32, isOutput=False)
    bg_d = nc.declare_dram_parameter("bg", [128, 1], F32, isOutput=False)
    bo_d = nc.declare_dram_parameter("bo", [128, 1], F32, isOutput=False)
    fc2_d = nc.declare_dram_parameter("fc2w", [HID, 1], F16, isOutput=False)
    out_d = nc.declare_dram_parameter("out", [1, BSIZE // NCORES], F32, isOutput=True)

    a1, a3, a5 = TANH_C5

    with tile.TileContext(nc) as tc:
        with (
            tc.tile_pool(name="const", bufs=1) as const,
            tc.tile_pool(name="decp", bufs=2) as decp,
            tc.tile_pool(name="work", bufs=2) as work,
            tc.tile_pool(name="psum", bufs=1, space="PSUM") as psum,
        ):
            wgt = {}
            for g, dr in [("i", wi_d), ("f", wf_d), ("g", wg_d), ("o", wo_d)]:
                wgt[g] = const.tile([128, HID], F16, tag=f"w{g}", name=f"w{g}")
                nc.sync.dma_start(out=wgt[g][:], in_=dr[:])
            bia = {}
            for g, dr in [("i", bi_d), ("f", bf_d), ("g", bg_d), ("o", bo_d)]:
                bia[g] = const.tile([128, 1], F32, tag=f"b{g}", name=f"b{g}")
                nc.sync.dma_start(out=bia[g][:], in_=dr[:])
            fc2 = const.tile([HID, 1], F16, tag="fc2", name="fc2")
            nc.sync.dma_start(out=fc2[:], in_=fc2_d[:])

            # ping-pong [x; h] tiles per half: rows 0:64 x_t, rows 64:128 h
            xh = [
                [
                    const.tile([128, HALF], F16, tag=f"xh{q}{p}", name=f"xh{q}{p}")
                    for p in range(2)
                ]
                for q in range(2)
            ]
            c2 = const.tile([128, HALF], F16, tag="c2", name="c2")
            nc.vector.memset(xh[0][0][HID:128, :], 0.0)
            nc.vector.memset(xh[1][0][HID:128, :], 0.0)
            nc.vector.memset(c2[:], 0.0)
            # x(0) into xh[*][0]
            nc.sync.dma_start(out=xh[0][0][0:DIM, :], in_=x_d[0, :, bass.ts(0, HALF)])
            nc.sync.dma_start(out=xh[1][0][0:DIM, :], in_=x_d[0, :, bass.ts(1, HALF)])
            dec0 = decp.tile([128, HALF], F16, tag="dec", name="dec0")
            nc.sync.dma_start(out=dec0[:], in_=dec_d[0])

            hfA = const.tile([HID, HALF], F16, tag="hfA", name="hfA")
            hfB = const.tile([HID, HALF], F16, tag="hfB", name="hfB")

            TAGS = ("sI", "tG", "sF", "sO", "dc", "ig", "fd", "tch", "tp", "pp")
            wrk = {}
            dect = {0: dec0}

            def emit_hmul(wp, parp, lastp, cd):
                ha = xh[0][1 - parp][HID:128, cd] if not lastp else hfA[:, cd]
                hb = xh[1][1 - parp][HID:128, cd] if not lastp else hfB[:, cd]
                nc.vector.tensor_mul(ha, wp["sO"][0:HID, cd], wp["tch"][0:HID, cd])
                nc.vector.tensor_mul(hb, wp["sO"][HID:128, cd], wp["tch"][HID:128, cd])

            def emit_mm(g, xa, xb, p, base):
                for j in range(CW // 512):
                    js = bass.ds(base + j * 512, 512)
                    ps = bass.ts(j, 512)
                    nc.tensor.matmul(
                        p[0:HID, ps], wgt[g][:], xa[:, js], start=True, stop=True
                    )
                for j in range(CW // 512):
                    js = bass.ds(base + j * 512, 512)
                    ps = bass.ts(j, 512)
                    nc.tensor.matmul(
                        p[HID:128, ps], wgt[g][:], xb[:, js], start=True, stop=True
                    )

            # software-pipelined half-step units: unit u=(s,L) computes lane
            # L's gates/c-update of step s and the *previous* unit's lane
            # tail (tanh(c) + h) so every cross-engine dependency has a full
            # unit of slack and the in-order ACT queue never stalls.
            for u in range(2 * STEP + 1):
                s, L = divmod(u, 2)
                Lp, sp = (1, s - 1) if L == 0 else (0, s)
                cur = s < STEP
                if cur and L == 0:
                    wk = {
                        tag: work.tile([128, HALF], F16, tag=tag, name=f"{tag}{s}")
                        for tag in TAGS
                    }
                    wrk[s % 2] = wk
                    if s + 1 < STEP:  # prefetch x(s+1), dec(s+1)
                        par1 = (s + 1) % 2
                        nc.sync.dma_start(
                            out=xh[0][par1][0:DIM, :],
                            in_=x_d[s + 1, :, bass.ts(0, HALF)],
                        )
                        nc.sync.dma_start(
                            out=xh[1][par1][0:DIM, :],
                            in_=x_d[s + 1, :, bass.ts(1, HALF)],
                        )
                        dn = decp.tile([128, HALF], F16, tag="dec", name=f"dec{s + 1}")
                        nc.sync.dma_start(out=dn[:], in_=dec_d[s + 1])
                        dect[(s + 1) % 2] = dn

                if cur:
                    wk = wrk[s % 2]
                    par = s % 2
                    xa, xb = xh[0][par], xh[1][par]
                    cs = bass.ds(L * CW, CW)
                    base = L * CW
                    nc.gpsimd.tensor_mul(
                        wk["dc"][:, cs], c2[:, cs], dect[s % 2][:, cs]
                    )
                    pI = psum.tile([128, CW], F32, tag="pi", name=f"pi{u}")
                    emit_mm("i", xa, xb, pI, base)
                    nc.scalar.activation(wk["sI"][:, cs], pI[:], AF.Sigmoid,
                                         bias=bia["i"][:])
                    pG = psum.tile([128, CW], F32, tag="pg", name=f"pg{u}")
                    emit_mm("g", xa, xb, pG, base)
                    nc.scalar.activation(wk["tG"][:, cs], pG[:], AF.Tanh,
                                         bias=bia["g"][:])
                    if IG_POOL:
                        nc.gpsimd.tensor_mul(wk["ig"][:, cs], wk["sI"][:, cs],
                                             wk["tG"][:, cs])
                    else:
                        nc.vector.tensor_mul(wk["ig"][:, cs], wk["sI"][:, cs],
                                             wk["tG"][:, cs])

                # previous unit's tail, part 1: ACT tanh(c) + its h-muls
                tail = 0 <= sp < STEP
                if tail:
                    wp = wrk[sp % 2]
                    parp = sp % 2
                    lastp = sp == STEP - 1
                    pbase = Lp * CW
                    wa = CW - TCW
                    if wa > 0:
                        ca = bass.ds(pbase, wa)
                        nc.scalar.activation(wp["tch"][:, ca], c2[:, ca], AF.Tanh)
                        for j0 in range(0, wa, 512):
                            emit_hmul(wp, parp, lastp,
                                      bass.ds(pbase + j0, min(512, wa - j0)))

                if cur:
                    pF = psum.tile([128, CW], F32, tag="pf", name=f"pf{u}")
                    emit_mm("f", xa, xb, pF, base)
                    nc.scalar.activation(wk["sF"][:, cs], pF[:], AF.Sigmoid,
                                         bias=bia["f"][:])

                # previous unit's tail, part 2: DVE tanh(c) chain + h-muls
                if tail and TCW > 0:
                    cd = bass.ds(pbase + wa, TCW)
                    tp_, pp_ = wp["tp"], wp["pp"]
                    nc.vector.tensor_mul(tp_[:, cd], c2[:, cd], c2[:, cd])
                    nc.vector.tensor_scalar(
                        out=pp_[:, cd], in0=tp_[:, cd],
                        scalar1=a5, scalar2=a3, op0=ALU.mult, op1=ALU.add,
                    )
                    nc.vector.tensor_mul(pp_[:, cd], pp_[:, cd], tp_[:, cd])
                    nc.vector.tensor_scalar(
                        out=pp_[:, cd], in0=pp_[:, cd],
                        scalar1=a1, scalar2=None, op0=ALU.add,
                    )
                    nc.vector.tensor_mul(wp["tch"][:, cd], pp_[:, cd], c2[:, cd])
                    for j0 in range(0, TCW, 512):
                        emit_hmul(wp, parp, lastp,
                                  bass.ds(pbase + wa + j0, min(512, TCW - j0)))

                if cur:
                    nc.vector.tensor_mul(wk["fd"][:, cs], wk["sF"][:, cs],
                                         wk["dc"][:, cs])
                    pO = psum.tile([128, CW], F32, tag="po", name=f"po{u}")
                    emit_mm("o", xa, xb, pO, base)
                    nc.scalar.activation(wk["sO"][:, cs], pO[:], AF.Sigmoid,
                                         bias=bia["o"][:])
                    nc.vector.tensor_add(c2[:, cs], wk["ig"][:, cs],
                                         wk["fd"][:, cs])

            # ---- final: q = 1 - sigmoid(h@w + b), noisy-OR over nodules ----
            nb2 = const.tile([1, 1], F32, tag="nb2", name="nb2")
            nc.vector.memset(nb2[:], -fc2_b)
            qall = const.tile([1, BL], F32, tag="qall", name="qall")
            for q, hf in ((0, hfA), (1, hfB)):
                for j in range(HALF // 512):
                    js = bass.ts(j, 512)
                    pz = psum.tile([1, 512], F32, tag="pi", name=f"pz{q}{j}")
                    nc.tensor.matmul(
                        pz[:], fc2[:], hf[:, js], start=True, stop=True
                    )
                    nc.scalar.activation(
                        qall[0:1, bass.ds(q * HALF + j * 512, 512)],
                        pz[:],
                        AF.Sigmoid,
                        scale=-1.0,
                        bias=nb2[:],
                    )
            # product over the 8 nodules (innermost in sample order)
            q3 = qall[0:1].rearrange("p (b n) -> p b n", n=NNOD)
            t1 = const.tile([1, BL // 2], F32, tag="t1", name="t1")
            t13 = t1[0:1].rearrange("p (b n) -> p b n", n=4)
            nc.vector.tensor_mul(t13[:, :, :], q3[:, :, 0:4], q3[:, :, 4:8])
            t2 = const.tile([1, BL // 4], F32, tag="t2", name="t2")
            t23 = t2[0:1].rearrange("p (b n) -> p b n", n=2)
            nc.vector.tensor_mul(t23[:, :, :], t13[:, :, 0:2], t13[:, :, 2:4])
            t3 = const.tile([1, BL // 8], F32, tag="t3", name="t3")
            t33 = t3[0:1].rearrange("p (b n) -> p b n", n=1)
            nc.vector.tensor_mul(t33[:, :, :], t23[:, :, 0:1], t23[:, :, 1:2])
            pred = const.tile([1, BSIZE // NCORES], F32, tag="pred", name="pred")
            nc.vector.tensor_scalar(
                out=pred[:],
                in0=t3[:],
                scalar1=-k_base,
                scalar2=1.0,
                op0=ALU.mult,
                op1=ALU.add,
            )
            nc.sync.dma_start(out=out_d[:], in_=pred[:])

    _split_multiwaits(nc)
    return nc


def kernel(input, time_dis, w_ih, w_hh, b_ih, b_hh, fc2_w, fc2_b, baseline):
    input = np.asarray(input, dtype=np.float32)
    time_dis = np.asarray(time_dis, dtype=np.float32)
    w_ih = np.asarray(w_ih, dtype=np.float32)
    w_hh = np.asarray(w_hh, dtype=np.float32)
    b_ih = np.asarray(b_ih, dtype=np.float32)
    b_hh = np.asarray(b_hh, dtype=np.float32)
    fc2_w = np.asarray(fc2_w, dtype=np.float32)
    fc2_b = np.asarray(fc2_b, dtype=np.float32)
    baseline = np.asarray(baseline, dtype=np.float32)

    f16 = np.float16
    bper = BSIZE // NCORES  # 512

    # gates^T = W^T.T @ [x;h], W = [w_ih | w_hh]  [256, 128]
    W = np.concatenate([w_ih, w_hh], axis=1)  # [256, 128]
    lhsT = np.ascontiguousarray(W.T)  # [128, 256] cols: i(0:64) f g o
    wi = np.ascontiguousarray(lhsT[:, 0:64]).astype(f16)
    wf = np.ascontiguousarray(lhsT[:, 64:128]).astype(f16)
    wg = np.ascontiguousarray(lhsT[:, 128:192]).astype(f16)
    wo = np.ascontiguousarray(lhsT[:, 192:256]).astype(f16)
    bias = (b_ih + b_hh).astype(np.float32)
    bi = np.ascontiguousarray(np.tile(bias[0:64], 2)[:, None])
    bfg = np.ascontiguousarray(np.tile(bias[64:128], 2)[:, None])
    bg = np.ascontiguousarray(np.tile(bias[128:192], 2)[:, None])
    bo = np.ascontiguousarray(np.tile(bias[192:256], 2)[:, None])
    fc2w = np.ascontiguousarray(fc2_w.reshape(1, HID).T).astype(f16)  # [64,1]
    k_base = float(1.0 - 1.0 / (1.0 + math.exp(-float(baseline[0]))))

    nc = _build(float(fc2_b[0]), k_base)

    in_maps = []
    for k in range(NCORES):
        bs = slice(k * bper, (k + 1) * bper)
        xs = input[:, bs].reshape(STEP, BL, DIM)
        xs = np.ascontiguousarray(xs.transpose(0, 2, 1)).astype(f16)  # [S,64,BL]
        td = time_dis[bs]  # [512, 32]
        td_bn = np.repeat(td.T, NNOD, axis=1)  # [32, 4096] sample-major
        td_used = np.concatenate([td_bn[:1], td_bn[:-1]], axis=0)
        dec = (1.0 / np.log(math.e + td_used)).astype(f16)  # [32, BL]
        # dec2[t, 0:64, j] = dec[t, j] (half A); [t, 64:128, j] = dec[t, HALF+j]
        dec2 = np.empty((STEP, 128, HALF), dtype=f16)
        dec2[:, 0:HID, :] = dec[:, None, 0:HALF]
        dec2[:, HID:128, :] = dec[:, None, HALF:BL]
        in_maps.append(
            {
                "x": xs,
                "dec": dec2,
                "wi": wi,
                "wf": wf,
                "wg": wg,
                "wo": wo,
                "bi": bi,
                "bf": bfg,
                "bg": bg,
                "bo": bo,
                "fc2w": fc2w,
            }
        )

    res = None
    last_err = None
    for _attempt in range(3):
        try:
            res = run_bass_kernel_spmd(nc, in_maps, list(range(NCORES)))
            break
        except Exception as e:  # transient NRT device errors recover on retry
            last_err = e
    if res is None:
        raise last_err
    global LAST_RESULT
    LAST_RESULT = res
    out = np.concatenate(
        [np.asarray(res.results[k]["out"]).reshape(bper) for k in range(NCORES)]
    )
    return out.astype(np.float32)
